# revision 32
# baseline (speedup 1.0000x reference)
"""Trainium2 Bass kernel for DGI (2x GCN + bilinear discriminator scores).

8-core SPMD, node-sharded, fp8 feature table:
  phase 1: per-core h = x @ W^T + b (bf16 matmul, bias as extra K-row),
           ACT-convert to fp8, store [node, h1|h2] rows (512B) in hcat
  phase 2: ONE AllGather -> rank-major replicated table hag [8*NPAD, 512] fp8
  phase 3: per 128-dest block: 4 quarter-gathers (2 ranks each, int16 local
           idx < 25088), one-hot S (bf16, DVE 4x) per 128-edge batch,
           4 chunk matmuls lhsT=G_chunk(fp8) rhs=S -> TRANSPOSED agg^T
           [h_chunk, dest] in PSUM; ACT PReLU -> resident SBUF hgT (bf16)
           with fused per-partition accum (GCN1 column sums)
  phase 3.5: AllReduce colsum -> s = sigmoid(mean); v = bilT @ s on PE
  phase 4: score columns via tiny matmuls lhsT=hgT_block rhs=vT -> one PSUM
           [128, 196]; + bias; host reassembles [1, 2N]

All edge structure (bucket/batch counts, slot maps) is computed on host from
the actual edge_index and baked into the (SPMD-uniform) program.
"""
import sys
sys.path.insert(0, '/opt/trn_rl_repo')
import numpy as np
import ml_dtypes

import concourse.bass as bass
import concourse.mybir as mybir
import concourse.tile as tile
from concourse import library_config
import bass_rust
from concourse.bass_utils import run_bass_kernel_spmd

N_CORES = 8
N_NODES = 100000
F = 512
H = 256
H2 = 2 * H
NPC = N_NODES // N_CORES          # 12500 nodes per core
NB = (NPC + 127) // 128           # 98 dest blocks per core
NPAD = NB * 128                   # 12544 padded nodes per core
P = 128
NQ = 4                            # gather buckets per block
QROWS = 2 * NPAD                  # 25088 rows per gather slice (< int16 max)
GN = 896                          # phase-1 node group (7 units)
NGRP = NPAD // GN                 # 14
AG_CHUNKS = 2                     # AllGather pipeline chunks (1 or 2)
CHR = NPAD // 2                   # source rows per AG chunk when chunked

f32 = mybir.dt.float32
bf16 = mybir.dt.bfloat16
fp8 = mybir.dt.float8e3      # e3m4: 4 mantissa bits; table pre-scaled by
HSCALE = 2.0                 # HSCALE to stay in normal range, descaled via
i16 = mybir.dt.int16         # the edge values baked into S

LAST_EXEC_NS = None

_CACHE = {}


def _split_multi_waits(nc, max_waits=1):
    """This walrus build only accepts one sync-wait per instruction; hoist
    extras onto preceding same-engine nops."""
    ctr = 0
    for bb in nc.main_func.blocks:
        new_list = []
        for ins in bb.instructions:
            si = ins.sync_info
            if si is not None and si.on_wait is not None and len(si.on_wait) > max_waits:
                waits = list(si.on_wait)
                while len(waits) > max_waits:
                    chunk, waits = waits[:max_waits], waits[max_waits:]
                    nop = mybir.InstNoOp(name=f"I-wsplit-{ctr}", ins=[], outs=[])
                    ctr += 1
                    nop.engine = ins.engine
                    nop.sync_info = bass_rust.SyncInfo(on_wait=chunk, on_update=[])
                    new_list.append(nop)
                ins.sync_info = bass_rust.SyncInfo(
                    on_wait=waits, on_update=list(si.on_update))
            new_list.append(ins)
        bb.instructions = new_list


def _wrap16(flat, ncols):
    """Pack a flat idx stream into the dma_gather [16, ncols] wrap, then
    replicate to 128 partitions (8 q7 cores)."""
    a = np.zeros((16, ncols), np.int16)
    n = len(flat)
    cols = (n + 15) // 16
    tmp = np.zeros(16 * cols, np.int16)
    tmp[:n] = flat
    a[:, :cols] = tmp.reshape(cols, 16).T
    return np.tile(a[None, :, :], (8, 1, 1)).reshape(P, ncols)


def _preprocess_edges(edge_index, edge_vals):
    """Bucket edges by (dest core, dest block, source quarter); pad each
    bucket to a multiple of 128 slots with (idx 0, ds 0, val 0). Batch counts
    are maxed across cores so the program is SPMD-uniform.

    Returns kb [NB, NQ], nbb [NB], TB, idx16 [8, P, TB*8] int16,
    meta_ds/meta_val [8, P, TB] f32.
    """
    row = np.asarray(edge_index[0]).astype(np.int64)
    col = np.asarray(edge_index[1]).astype(np.int64)
    val = np.asarray(edge_vals, dtype=np.float32)

    c = row // NPC
    r = (row - c * NPC).astype(np.int32)
    ds = (r & 127).astype(np.float32)
    rk = col // NPC
    l = (col - rk * NPC).astype(np.int32)
    if AG_CHUNKS == 1:
        q = rk >> 1
        grow = ((rk & 1) * NPAD + l).astype(np.int16)   # row within quarter
    else:
        ch = l // CHR
        q = ch * 2 + (rk >> 2)
        grow = ((rk & 3) * CHR + (l - ch * CHR)).astype(np.int16)
    gb = (c * NB + (r >> 7)) * NQ + q                    # global bucket id

    order = np.argsort(gb, kind="stable")
    gb_s = gb[order]
    cnt = np.bincount(gb, minlength=N_CORES * NB * NQ).reshape(N_CORES, NB, NQ)
    kb = -(-cnt.max(axis=0) // 128)                      # [NB, NQ]
    if AG_CHUNKS == 1:
        zero_blocks = kb.sum(axis=1) == 0
        kb[zero_blocks, 0] = 1
    else:
        # each half-sweep must touch every block (partial copy + final PReLU)
        kb[kb[:, 0] + kb[:, 1] == 0, 0] = 1
        kb[kb[:, 2] + kb[:, 3] == 0, 2] = 1
    nbb = kb.sum(axis=1)                                 # [NB]
    TB = int(nbb.sum())

    boff = np.zeros(NB * NQ + 1, np.int64)
    np.cumsum(kb.ravel(), out=boff[1:])                  # batch offset of bucket
    SLOTS = TB * P

    first = np.searchsorted(gb_s, np.arange(N_CORES * NB * NQ))
    pos = np.arange(len(gb_s)) - first[gb_s]             # pos within bucket
    lb = gb_s % (NB * NQ)
    slot = boff[lb] * P + pos
    core_s = gb_s // (NB * NQ)

    idx_all = np.zeros((N_CORES, SLOTS), np.int16)
    ds_all = np.zeros((N_CORES, SLOTS), np.float32)
    val_all = np.zeros((N_CORES, SLOTS), np.float32)
    idx_all[core_s, slot] = grow[order]
    ds_all[core_s, slot] = ds[order]
    val_all[core_s, slot] = val[order] / HSCALE

    idx16 = np.stack([_wrap16(idx_all[cc], TB * 8) for cc in range(N_CORES)])
    meta_ds = np.ascontiguousarray(
        ds_all.reshape(N_CORES, TB, P).transpose(0, 2, 1))
    meta_val = np.ascontiguousarray(
        val_all.reshape(N_CORES, TB, P).transpose(0, 2, 1))
    return kb, nbb, TB, idx16, meta_ds, meta_val


def _build_program(kb, nbb, TB, with_bias=True, lower=True, debug_outs=False):
    nc = bass.Bass("TRN2", target_bir_lowering=False, debug=False,
                   num_devices=N_CORES)

    # ---- I/O ----
    xT_in = nc.dram_tensor("xT", [2, F, NPAD], bf16, kind="ExternalInput")
    wT_in = nc.dram_tensor("wT", [F, H], bf16, kind="ExternalInput")
    fcb_in = nc.dram_tensor("fcb", [1, H], bf16, kind="ExternalInput")
    alpha_in = nc.dram_tensor("alpha", [1], f32, kind="ExternalInput")
    bilT_in = nc.dram_tensor("bilT", [H, H], f32, kind="ExternalInput")
    bilb_in = nc.dram_tensor("bilb", [1], f32, kind="ExternalInput")
    iota_in = nc.dram_tensor("iota", [P], bf16, kind="ExternalInput")
    idx_in = nc.dram_tensor("idx16", [P, TB * 8], i16, kind="ExternalInput")
    mds_in = nc.dram_tensor("mds", [P, TB], f32, kind="ExternalInput")
    mval_in = nc.dram_tensor("mval", [P, TB], f32, kind="ExternalInput")
    score_out = nc.dram_tensor("scores", [2, P, NB], f32, kind="ExternalOutput")
    if debug_outs:
        hcat_out = nc.dram_tensor("hcat_o", [NPAD, H2], fp8, kind="ExternalOutput")
        hag_out = nc.dram_tensor("hag_o", [4 * P, H2], fp8, kind="ExternalOutput")
        hgt_out = nc.dram_tensor("hgt_o", [P, NB * H2], bf16, kind="ExternalOutput")
        acc_out = nc.dram_tensor("acc_o", [P, 2 * NB], f32, kind="ExternalOutput")
        gt_out = nc.dram_tensor("gt_o", [P, 4 * H2], fp8, kind="ExternalOutput")

    bstart = np.zeros(NB + 1, np.int64)
    bstart[1:] = np.cumsum(nbb)

    with tile.TileContext(nc) as tc:
        with tc.tile_pool(name="const", bufs=1) as cpool, \
             tc.tile_pool(name="x", bufs=2) as xpool, \
             tc.tile_pool(name="meta", bufs=1) as mpool, \
             tc.tile_pool(name="idxp", bufs=3) as ipool, \
             tc.tile_pool(name="g", bufs=6) as gpool, \
             tc.tile_pool(name="s", bufs=8) as spool, \
             tc.tile_pool(name="h", bufs=3) as hpool, \
             tc.tile_pool(name="res", bufs=1) as rpool, \
             tc.tile_pool(name="ps", bufs=2, space="PSUM") as psp, \
             tc.tile_pool(name="dram", bufs=1, space="DRAM") as dpool:

            nc.gpsimd.load_library(library_config.mlp)

            # ---- internal DRAM ----
            hcat = dpool.tile([NPAD, H2], fp8)
            if AG_CHUNKS == 1:
                ag_bufs = [dpool.tile([N_CORES * NPAD, H2], fp8,
                                      addr_space="Shared")]
            else:
                ag_bufs = [dpool.tile([N_CORES * CHR, H2], fp8,
                                      addr_space="Shared", name=f"ag{i}")
                           for i in range(2)]
            cs_in = dpool.tile([P, 2], f32)
            cs_out = dpool.tile([P, 2], f32, addr_space="Shared")

            # ---- constants ----
            wT_t = cpool.tile([P, 4 * H], bf16)
            for fc in range(4):
                nc.sync.dma_start(out=wT_t[:, fc * H:(fc + 1) * H],
                                  in_=wT_in[fc * P:(fc + 1) * P, :])
            fcb_t = cpool.tile([1, H], bf16)
            nc.sync.dma_start(out=fcb_t[:], in_=fcb_in[:])
            ones_t = cpool.tile([1, P], bf16)
            nc.vector.memset(ones_t[:], 1.0)
            alpha_t = cpool.tile([P, 1], f32)
            nc.sync.dma_start(out=alpha_t[:], in_=alpha_in[None, :].to_broadcast((P, 1)))
            iota_t = cpool.tile([P, P], bf16)
            nc.sync.dma_start(out=iota_t[:], in_=iota_in[None, :].to_broadcast((P, P)))
            bilT_t = [cpool.tile([P, H], f32, tag=f"bilT{gc}", name=f"bilT{gc}")
                      for gc in range(2)]
            for gc in range(2):
                nc.sync.dma_start(out=bilT_t[gc][:],
                                  in_=bilT_in[gc * P:(gc + 1) * P, :])
            bilb_t = cpool.tile([P, 1], f32)
            nc.sync.dma_start(out=bilb_t[:],
                              in_=bilb_in[None, :].to_broadcast((P, 1)))

            # ---- residents ----
            hgT = rpool.tile([P, NB * H2], bf16)     # agg^T, PReLU'd, bf16
            accA = rpool.tile([P, 2 * NB], f32)      # GCN1 colsums per block

            # ---- phase 1 + 2: h = x @ W^T + b -> fp8 hcat rows; chunked
            # rank-major AllGather fired as soon as its source rows exist ----
            for agc in range(AG_CHUNKS):
                for g in range(agc * NGRP // AG_CHUNKS,
                               (agc + 1) * NGRP // AG_CHUNKS):
                    xg = [xpool.tile([P, 4, GN], bf16, tag=f"xg{gcn}",
                                     name=f"xg{gcn}_{g}") for gcn in range(2)]
                    for gcn in range(2):
                        nc.sync.dma_start(
                            out=xg[gcn][:],
                            in_=xT_in[gcn].rearrange("(c p) n -> p c n", p=P)
                                [:, :, g * GN:(g + 1) * GN])
                    for sub in range(GN // P):
                        ht2 = hpool.tile([P, H2], fp8, tag="ht2")
                        for gcn in range(2):
                            hp = psp.tile([P, H], f32, space="PSUM",
                                          tag=f"b{gcn}")
                            for fc in range(4):
                                nc.tensor.matmul(
                                    hp[:],
                                    lhsT=xg[gcn][:, fc, sub * P:(sub + 1) * P],
                                    rhs=wT_t[:, fc * H:(fc + 1) * H],
                                    start=(fc == 0),
                                    stop=(fc == 3 and not with_bias))
                            if with_bias:
                                nc.tensor.matmul(hp[:], lhsT=ones_t[:1, :],
                                                 rhs=fcb_t[:1, :],
                                                 start=False, stop=True)
                            nc.scalar.activation(
                                out=ht2[:, gcn * H:(gcn + 1) * H], in_=hp[:],
                                func=mybir.ActivationFunctionType.Copy,
                                scale=HSCALE)
                        n0 = (g * (GN // P) + sub) * P
                        nc.sync.dma_start(out=hcat[n0:n0 + P, :], in_=ht2[:])
                nrows = NPAD // AG_CHUNKS
                nc.gpsimd.collective_compute(
                    "AllGather", mybir.AluOpType.bypass,
                    ins=[hcat[agc * nrows:(agc + 1) * nrows, :].opt()],
                    outs=[ag_bufs[agc][:].opt()],
                    replica_groups=[list(range(N_CORES))])

            # ---- metadata (resident) ----
            mds_t = mpool.tile([P, TB], f32)
            nc.sync.dma_start(out=mds_t[:], in_=mds_in[:])
            mval_t = mpool.tile([P, TB], f32)
            nc.sync.dma_start(out=mval_t[:], in_=mval_in[:])

            nreg_cache = {}

            def count_reg(v):
                if v not in nreg_cache:
                    nreg_cache[v] = nc.gpsimd.to_reg(v)
                return nreg_cache[v]

            # ---- phase 3: transposed aggregation ----
            # sweeps[i] = (q_lo, q_hi, mode): "full" closes the block in one
            # pass; "partial" parks raw agg^T in hgT; "final" adds the parked
            # partial back and applies PReLU.
            if AG_CHUNKS == 1:
                sweeps = [(0, NQ, "full")]
            else:
                sweeps = [(0, 2, "partial"), (2, 4, "final")]
            for q_lo, q_hi, mode in sweeps:
                for b in range(NB):
                    s0 = int(bstart[b])
                    koff0 = int(kb[b, :q_lo].sum())
                    nbatch = int(kb[b, q_lo:q_hi].sum())
                    if nbatch == 0:
                        continue
                    m0 = s0 + koff0           # first meta/batch column
                    it = ipool.tile([P, nbatch * 8], i16, tag="idx",
                                    name=f"idx{mode[0]}{b}")
                    nc.sync.dma_start(out=it[:],
                                      in_=idx_in[:, m0 * 8:(m0 + nbatch) * 8])
                    buckets = []          # (tile, koff, kbr)
                    koff = 0
                    for qq in range(q_lo, q_hi):
                        kbr = int(kb[b, qq])
                        if kbr == 0:
                            continue
                        if AG_CHUNKS == 1:
                            src = ag_bufs[0][qq * QROWS:(qq + 1) * QROWS, :]
                        else:
                            src = ag_bufs[qq >> 1][(qq & 1) * QROWS:
                                                   ((qq & 1) + 1) * QROWS, :]
                        gt = gpool.tile([P, kbr * H2], fp8, tag="g",
                                        name=f"g{mode[0]}{b}_{qq}")
                        nc.gpsimd.dma_gather(
                            out_ap=gt[:].rearrange("p (k h) -> p k h", k=kbr),
                            in_ap=src,
                            idxs_ap=it[:, koff * 8:(koff + kbr) * 8],
                            num_idxs=kbr * P,
                            num_idxs_reg=count_reg(kbr * P),
                            elem_size=H2,
                            single_packet=False)
                        if debug_outs and b == 1 and AG_CHUNKS == 1:
                            nc.sync.dma_start(
                                out=gt_out[:, qq * H2:(qq + 1) * H2],
                                in_=gt[:, :H2])
                        buckets.append((gt, koff, kbr))
                        koff += kbr
                    pT = [psp.tile([P, P], f32, space="PSUM", tag=f"b{cch}",
                                   name=f"pT{mode[0]}{b}_{cch}")
                          for cch in range(4)]
                    for gt, koff, kbr in buckets:
                        for jj in range(kbr):
                            j = koff + jj
                            s_t = spool.tile([P, P], bf16, tag="s")
                            nc.vector.tensor_scalar(
                                out=s_t[:], in0=iota_t[:],
                                scalar1=mds_t[:, m0 + j:m0 + j + 1],
                                scalar2=mval_t[:, m0 + j:m0 + j + 1],
                                op0=mybir.AluOpType.is_equal,
                                op1=mybir.AluOpType.mult)
                            for cch in range(4):
                                nc.tensor.matmul(
                                    pT[cch][:],
                                    lhsT=gt[:, jj * H2 + cch * P:
                                            jj * H2 + (cch + 1) * P],
                                    rhs=s_t[:],
                                    start=(j == 0), stop=(j == nbatch - 1))
                    for cch in range(4):
                        dst = hgT[:, b * H2 + cch * P:b * H2 + (cch + 1) * P]
                        if mode == "partial":
                            nc.scalar.activation(
                                out=dst, in_=pT[cch][:],
                                func=mybir.ActivationFunctionType.Copy)
                            continue
                        if mode == "final":
                            sum_t = hpool.tile([P, P], bf16, tag="sum")
                            nc.vector.tensor_tensor(
                                out=sum_t[:], in0=pT[cch][:], in1=dst,
                                op=mybir.AluOpType.add)
                            src_ap = sum_t[:]
                        else:
                            src_ap = pT[cch][:]
                        kwargs = {}
                        if cch < 2:
                            kwargs["accum_out"] = accA[:, cch * NB + b:
                                                       cch * NB + b + 1]
                        nc.scalar.activation(
                            out=dst, in_=src_ap,
                            func=mybir.ActivationFunctionType.Prelu,
                            alpha=alpha_t[:, :1], **kwargs)

            # ---- phase 3.5: s = sigmoid(mean(h1_gcn)); v = bilT @ s ----
            cs_t = hpool.tile([P, 2], f32, tag="cs")
            for cch in range(2):
                nc.vector.tensor_reduce(
                    out=cs_t[:, cch:cch + 1],
                    in_=accA[:, cch * NB:(cch + 1) * NB],
                    axis=mybir.AxisListType.X, op=mybir.AluOpType.add)
            nc.sync.dma_start(out=cs_in[:], in_=cs_t[:])
            nc.gpsimd.collective_compute(
                "AllReduce", mybir.AluOpType.add,
                ins=[cs_in[:].opt()], outs=[cs_out[:].opt()],
                replica_groups=[list(range(N_CORES))])
            cso_t = hpool.tile([P, 2], f32, tag="cso")
            nc.sync.dma_start(out=cso_t[:], in_=cs_out[:])
            sT_t = hpool.tile([P, 2], f32, tag="sT")
            nc.scalar.activation(out=sT_t[:], in_=cso_t[:],
                                 func=mybir.ActivationFunctionType.Sigmoid,
                                 scale=1.0 / N_NODES)
            vp = psp.tile([P, 2], f32, space="PSUM", tag="b3")
            for hc in range(2):
                for gc in range(2):
                    nc.tensor.matmul(
                        vp[:, hc:hc + 1],
                        lhsT=bilT_t[gc][:, hc * P:(hc + 1) * P],
                        rhs=sT_t[:, gc:gc + 1],
                        start=(gc == 0), stop=(gc == 1))
            vT_t = hpool.tile([P, 2], bf16, tag="vT")
            nc.vector.tensor_copy(out=vT_t[:], in_=vp[:])

            # ---- phase 4: scores via tiny matmuls into one PSUM ----
            psc = psp.tile([P, 2 * NB], f32, space="PSUM", tag="b2")
            for g2 in range(2):
                for b in range(NB):
                    for hc in range(2):
                        nc.tensor.matmul(
                            psc[:, g2 * NB + b:g2 * NB + b + 1],
                            lhsT=hgT[:, b * H2 + (2 * g2 + hc) * P:
                                     b * H2 + (2 * g2 + hc + 1) * P],
                            rhs=vT_t[:, hc:hc + 1],
                            start=(hc == 0), stop=(hc == 1))
            if debug_outs:
                nc.sync.dma_start(out=hcat_out[:], in_=hcat[:])
                if AG_CHUNKS == 1:
                    for qq in range(4):
                        nc.sync.dma_start(
                            out=hag_out[qq * P:(qq + 1) * P, :],
                            in_=ag_bufs[0][qq * QROWS + 512:
                                           qq * QROWS + 512 + P, :])
                nc.sync.dma_start(out=hgt_out[:], in_=hgT[:])
                nc.sync.dma_start(out=acc_out[:], in_=accA[:])

            sc_t = hpool.tile([P, 2 * NB], f32, tag="scb")
            nc.vector.tensor_scalar(
                out=sc_t[:], in0=psc[:], scalar1=bilb_t[:, :1],
                scalar2=None, op0=mybir.AluOpType.add)
            for g2 in range(2):
                nc.sync.dma_start(out=score_out[g2],
                                  in_=sc_t[:, g2 * NB:(g2 + 1) * NB])

    if lower:
        mybir.codegen_inst_isa_subclasses(nc)
        _split_multi_waits(nc)
    return nc


def kernel(x_1, x_2, edge_vals, fc_w, fc_b, prelu_a, bil_w, bil_b, edge_index):
    global LAST_EXEC_NS
    import hashlib
    ek = (hashlib.blake2b(np.ascontiguousarray(edge_index).tobytes(),
                          digest_size=16).digest(),
          hashlib.blake2b(np.ascontiguousarray(edge_vals).tobytes(),
                          digest_size=16).digest())
    if _CACHE.get("ekey") != ek:
        _CACHE.clear()
        _CACHE["ekey"] = ek
        _CACHE["pre"] = _preprocess_edges(edge_index, edge_vals)
    kb, nbb, TB, idx16, meta_ds, meta_val = _CACHE["pre"]

    with_bias = bool(np.any(np.asarray(fc_b)))
    pkey = ("prog", TB, with_bias, kb.tobytes())
    if pkey not in _CACHE:
        _CACHE[pkey] = _build_program(kb, nbb, TB, with_bias=with_bias)
    nc = _CACHE[pkey]

    bf = ml_dtypes.bfloat16
    x1 = np.asarray(x_1, np.float32).reshape(N_NODES, F)
    x2 = np.asarray(x_2, np.float32).reshape(N_NODES, F)
    wT = np.ascontiguousarray(np.asarray(fc_w, np.float32).T).astype(bf)
    bilT = np.ascontiguousarray(np.asarray(bil_w, np.float32)[0].T)
    fcb = np.asarray(fc_b, np.float32).reshape(1, H).astype(bf)
    iota = np.arange(P, dtype=np.float32).astype(bf)

    in_maps = []
    for c in range(N_CORES):
        xs = np.zeros((2, F, NPAD), bf)
        xs[0, :, :NPC] = x1[c * NPC:(c + 1) * NPC].T.astype(bf)
        xs[1, :, :NPC] = x2[c * NPC:(c + 1) * NPC].T.astype(bf)
        in_maps.append({
            "xT": xs,
            "wT": wT,
            "fcb": fcb,
            "alpha": np.asarray(prelu_a, np.float32).reshape(1),
            "bilT": bilT,
            "bilb": np.asarray(bil_b, np.float32).reshape(1),
            "iota": iota,
            "idx16": idx16[c],
            "mds": meta_ds[c],
            "mval": meta_val[c],
        })

    res = run_bass_kernel_spmd(nc, in_maps, list(range(N_CORES)))
    if res.exec_time_ns is not None:
        LAST_EXEC_NS = res.exec_time_ns

    out = np.empty((1, 2 * N_NODES), np.float32)
    for c in range(N_CORES):
        sc = res.results[c]["scores"]          # [2, 128, NB]
        out[0, c * NPC:(c + 1) * NPC] = sc[0].T.ravel()[:NPC]
        out[0, N_NODES + c * NPC:N_NODES + (c + 1) * NPC] = sc[1].T.ravel()[:NPC]
    return out


# revision 35
# speedup vs baseline: 8538.9419x; 8538.9419x over previous
"""Trainium2 Bass kernel for DGI (2x GCN + bilinear discriminator scores).

8-core SPMD, node-sharded, fp8(e3m4, x2-scaled) feature table:
  phase 1: per-core h = x @ W^T + b (bf16 matmul, bias as extra K-row),
           ACT-convert to fp8 (scale folded out via edge vals), store
           [node, h1|h2] rows (512B) in hcat
  phase 2: rank-major AllGather of the table, in AG_CHUNKS source-row
           chunks so the second chunk's transfer hides under phase 3
  phase 3: per 128-dest block and AG chunk: int16 row-sliced gathers
           (<25088 rows per slice), one-hot S (bf16, DVE 4x) per 128-edge
           batch, 4 chunk matmuls lhsT=G_chunk(fp8) rhs=S -> TRANSPOSED
           agg^T [h_chunk, dest], one PSUM bank per chunk (2KB zero
           regions!); chunk-0 partials parked raw in hgT, summed back in
           the final sweep; ACT PReLU -> resident SBUF hgT (bf16) with
           fused per-partition accum (GCN1 column sums)
  phase 3.5: AllReduce colsum -> s = sigmoid(mean); v = bilT @ s on PE
  phase 4: score columns via tiny matmuls lhsT=hgT_block rhs=vT -> one PSUM
           [128, 196]; + bias; host reassembles [1, 2N]

All edge structure (bucket/batch counts, slot maps) is computed on host from
the actual edge_index and baked into the (SPMD-uniform) program.
"""
import sys
sys.path.insert(0, '/opt/trn_rl_repo')
import numpy as np
import ml_dtypes

import concourse.bass as bass
import concourse.mybir as mybir
import concourse.tile as tile
from concourse import library_config
import bass_rust
from concourse.bass_utils import run_bass_kernel_spmd

N_CORES = 8
N_NODES = 100000
F = 512
H = 256
H2 = 2 * H
NPC = N_NODES // N_CORES          # 12500 nodes per core
NB = (NPC + 127) // 128           # 98 dest blocks per core
NPAD = NB * 128                   # 12544 padded nodes per core
P = 128
NQ = 4                            # gather buckets per block
QROWS = 2 * NPAD                  # 25088 rows per gather slice (< int16 max)
GN = 896                          # phase-1 node group (7 units)
NGRP = NPAD // GN                 # 14
AG_CHUNKS = 2                     # AllGather pipeline chunks (1 or 2)
CHR = NPAD // 2                   # source rows per AG chunk when chunked

f32 = mybir.dt.float32
bf16 = mybir.dt.bfloat16
fp8 = mybir.dt.float8e3      # e3m4: 4 mantissa bits; table pre-scaled by
HSCALE = 2.0                 # HSCALE to stay in normal range, descaled via
i16 = mybir.dt.int16         # the edge values baked into S

LAST_EXEC_NS = None
LAST_PROGRAM = None

_CACHE = {}


def _split_multi_waits(nc, max_waits=1):
    """This walrus build only accepts one sync-wait per instruction; hoist
    extras onto preceding same-engine nops."""
    ctr = 0
    for bb in nc.main_func.blocks:
        new_list = []
        for ins in bb.instructions:
            si = ins.sync_info
            if si is not None and si.on_wait is not None and len(si.on_wait) > max_waits:
                waits = list(si.on_wait)
                while len(waits) > max_waits:
                    chunk, waits = waits[:max_waits], waits[max_waits:]
                    nop = mybir.InstNoOp(name=f"I-wsplit-{ctr}", ins=[], outs=[])
                    ctr += 1
                    nop.engine = ins.engine
                    nop.sync_info = bass_rust.SyncInfo(on_wait=chunk, on_update=[])
                    new_list.append(nop)
                ins.sync_info = bass_rust.SyncInfo(
                    on_wait=waits, on_update=list(si.on_update))
            new_list.append(ins)
        bb.instructions = new_list


def _wrap16(flat, ncols):
    """Pack a flat idx stream into the dma_gather [16, ncols] wrap, then
    replicate to 128 partitions (8 q7 cores)."""
    a = np.zeros((16, ncols), np.int16)
    n = len(flat)
    cols = (n + 15) // 16
    tmp = np.zeros(16 * cols, np.int16)
    tmp[:n] = flat
    a[:, :cols] = tmp.reshape(cols, 16).T
    return np.tile(a[None, :, :], (8, 1, 1)).reshape(P, ncols)


def _preprocess_edges(edge_index, edge_vals):
    """Bucket edges by (dest core, dest block, source quarter); pad each
    bucket to a multiple of 128 slots with (idx 0, ds 0, val 0). Batch counts
    are maxed across cores so the program is SPMD-uniform.

    Returns kb [NB, NQ], nbb [NB], TB, idx16 [8, P, TB*8] int16,
    meta_ds/meta_val [8, P, TB] f32.
    """
    row = np.asarray(edge_index[0]).astype(np.int64)
    col = np.asarray(edge_index[1]).astype(np.int64)
    val = np.asarray(edge_vals, dtype=np.float32)

    c = row // NPC
    r = (row - c * NPC).astype(np.int32)
    ds = (r & 127).astype(np.float32)
    rk = col // NPC
    l = (col - rk * NPC).astype(np.int32)
    if AG_CHUNKS == 1:
        q = rk >> 1
        grow = ((rk & 1) * NPAD + l).astype(np.int16)   # row within quarter
    else:
        ch = l // CHR
        q = ch * 2 + (rk >> 2)
        grow = ((rk & 3) * CHR + (l - ch * CHR)).astype(np.int16)
    gb = (c * NB + (r >> 7)) * NQ + q                    # global bucket id

    order = np.argsort(gb, kind="stable")
    gb_s = gb[order]
    cnt = np.bincount(gb, minlength=N_CORES * NB * NQ).reshape(N_CORES, NB, NQ)
    kb = -(-cnt.max(axis=0) // 128)                      # [NB, NQ]
    if AG_CHUNKS == 1:
        zero_blocks = kb.sum(axis=1) == 0
        kb[zero_blocks, 0] = 1
    else:
        # each half-sweep must touch every block (partial copy + final PReLU)
        kb[kb[:, 0] + kb[:, 1] == 0, 0] = 1
        kb[kb[:, 2] + kb[:, 3] == 0, 2] = 1
    nbb = kb.sum(axis=1)                                 # [NB]
    TB = int(nbb.sum())

    boff = np.zeros(NB * NQ + 1, np.int64)
    np.cumsum(kb.ravel(), out=boff[1:])                  # batch offset of bucket
    SLOTS = TB * P

    first = np.searchsorted(gb_s, np.arange(N_CORES * NB * NQ))
    pos = np.arange(len(gb_s)) - first[gb_s]             # pos within bucket
    lb = gb_s % (NB * NQ)
    slot = boff[lb] * P + pos
    core_s = gb_s // (NB * NQ)

    idx_all = np.zeros((N_CORES, SLOTS), np.int16)
    ds_all = np.zeros((N_CORES, SLOTS), np.float32)
    val_all = np.zeros((N_CORES, SLOTS), np.float32)
    idx_all[core_s, slot] = grow[order]
    ds_all[core_s, slot] = ds[order]
    val_all[core_s, slot] = val[order] / HSCALE

    idx16 = np.stack([_wrap16(idx_all[cc], TB * 8) for cc in range(N_CORES)])
    meta_ds = np.ascontiguousarray(
        ds_all.reshape(N_CORES, TB, P).transpose(0, 2, 1))
    meta_val = np.ascontiguousarray(
        val_all.reshape(N_CORES, TB, P).transpose(0, 2, 1))
    return kb, nbb, TB, idx16, meta_ds, meta_val


def _build_program(kb, nbb, TB, with_bias=True, lower=True, debug_outs=False):
    nc = bass.Bass("TRN2", target_bir_lowering=False, debug=False,
                   num_devices=N_CORES)

    # ---- I/O ----
    xT_in = nc.dram_tensor("xT", [2, F, NPAD], bf16, kind="ExternalInput")
    wT_in = nc.dram_tensor("wT", [F, H], bf16, kind="ExternalInput")
    fcb_in = nc.dram_tensor("fcb", [1, H], bf16, kind="ExternalInput")
    alpha_in = nc.dram_tensor("alpha", [1], f32, kind="ExternalInput")
    bilT_in = nc.dram_tensor("bilT", [H, H], f32, kind="ExternalInput")
    bilb_in = nc.dram_tensor("bilb", [1], f32, kind="ExternalInput")
    iota_in = nc.dram_tensor("iota", [P], bf16, kind="ExternalInput")
    idx_in = nc.dram_tensor("idx16", [P, TB * 8], i16, kind="ExternalInput")
    mds_in = nc.dram_tensor("mds", [P, TB], f32, kind="ExternalInput")
    mval_in = nc.dram_tensor("mval", [P, TB], f32, kind="ExternalInput")
    score_out = nc.dram_tensor("scores", [2, P, NB], f32, kind="ExternalOutput")
    if debug_outs:
        hcat_out = nc.dram_tensor("hcat_o", [NPAD, H2], fp8, kind="ExternalOutput")
        hag_out = nc.dram_tensor("hag_o", [4 * P, H2], fp8, kind="ExternalOutput")
        hgt_out = nc.dram_tensor("hgt_o", [P, NB * H2], bf16, kind="ExternalOutput")
        acc_out = nc.dram_tensor("acc_o", [P, 2 * NB], f32, kind="ExternalOutput")
        gt_out = nc.dram_tensor("gt_o", [P, 4 * H2], fp8, kind="ExternalOutput")

    bstart = np.zeros(NB + 1, np.int64)
    bstart[1:] = np.cumsum(nbb)

    with tile.TileContext(nc) as tc:
        with tc.tile_pool(name="const", bufs=1) as cpool, \
             tc.tile_pool(name="x", bufs=2) as xpool, \
             tc.tile_pool(name="meta", bufs=1) as mpool, \
             tc.tile_pool(name="idxp", bufs=3) as ipool, \
             tc.tile_pool(name="g", bufs=6) as gpool, \
             tc.tile_pool(name="s", bufs=8) as spool, \
             tc.tile_pool(name="h", bufs=3) as hpool, \
             tc.tile_pool(name="res", bufs=1) as rpool, \
             tc.tile_pool(name="ps", bufs=2, space="PSUM") as psp, \
             tc.tile_pool(name="dram", bufs=1, space="DRAM") as dpool:

            nc.gpsimd.load_library(library_config.mlp)

            # ---- internal DRAM ----
            hcat = dpool.tile([NPAD, H2], fp8)
            if AG_CHUNKS == 1:
                ag_bufs = [dpool.tile([N_CORES * NPAD, H2], fp8,
                                      addr_space="Shared")]
            else:
                ag_bufs = [dpool.tile([N_CORES * CHR, H2], fp8,
                                      addr_space="Shared", name=f"ag{i}")
                           for i in range(2)]
            cs_in = dpool.tile([P, 2], f32)
            cs_out = dpool.tile([P, 2], f32, addr_space="Shared")

            # ---- constants ----
            wT_t = cpool.tile([P, 4 * H], bf16)
            for fc in range(4):
                nc.sync.dma_start(out=wT_t[:, fc * H:(fc + 1) * H],
                                  in_=wT_in[fc * P:(fc + 1) * P, :])
            fcb_t = cpool.tile([1, H], bf16)
            nc.sync.dma_start(out=fcb_t[:], in_=fcb_in[:])
            ones_t = cpool.tile([1, P], bf16)
            nc.vector.memset(ones_t[:], 1.0)
            alpha_t = cpool.tile([P, 1], f32)
            nc.sync.dma_start(out=alpha_t[:], in_=alpha_in[None, :].to_broadcast((P, 1)))
            iota_t = cpool.tile([P, P], bf16)
            nc.sync.dma_start(out=iota_t[:], in_=iota_in[None, :].to_broadcast((P, P)))
            bilT_t = [cpool.tile([P, H], f32, tag=f"bilT{gc}", name=f"bilT{gc}")
                      for gc in range(2)]
            for gc in range(2):
                nc.sync.dma_start(out=bilT_t[gc][:],
                                  in_=bilT_in[gc * P:(gc + 1) * P, :])
            bilb_t = cpool.tile([P, 1], f32)
            nc.sync.dma_start(out=bilb_t[:],
                              in_=bilb_in[None, :].to_broadcast((P, 1)))

            # ---- residents ----
            hgT = rpool.tile([P, NB * H2], bf16)     # agg^T, PReLU'd, bf16
            accA = rpool.tile([P, 2 * NB], f32)      # GCN1 colsums per block

            # ---- phase 1 + 2: h = x @ W^T + b -> fp8 hcat rows; chunked
            # rank-major AllGather fired as soon as its source rows exist ----
            for agc in range(AG_CHUNKS):
                for g in range(agc * NGRP // AG_CHUNKS,
                               (agc + 1) * NGRP // AG_CHUNKS):
                    xg = [xpool.tile([P, 4, GN], bf16, tag=f"xg{gcn}",
                                     name=f"xg{gcn}_{g}") for gcn in range(2)]
                    for gcn in range(2):
                        nc.sync.dma_start(
                            out=xg[gcn][:],
                            in_=xT_in[gcn].rearrange("(c p) n -> p c n", p=P)
                                [:, :, g * GN:(g + 1) * GN])
                    for sub in range(GN // P):
                        ht2 = hpool.tile([P, H2], fp8, tag="ht2")
                        for gcn in range(2):
                            hp = psp.tile([P, H], f32, space="PSUM",
                                          tag=f"b{gcn}")
                            for fc in range(4):
                                nc.tensor.matmul(
                                    hp[:],
                                    lhsT=xg[gcn][:, fc, sub * P:(sub + 1) * P],
                                    rhs=wT_t[:, fc * H:(fc + 1) * H],
                                    start=(fc == 0),
                                    stop=(fc == 3 and not with_bias))
                            if with_bias:
                                nc.tensor.matmul(hp[:], lhsT=ones_t[:1, :],
                                                 rhs=fcb_t[:1, :],
                                                 start=False, stop=True)
                            nc.scalar.activation(
                                out=ht2[:, gcn * H:(gcn + 1) * H], in_=hp[:],
                                func=mybir.ActivationFunctionType.Copy,
                                scale=HSCALE)
                        n0 = (g * (GN // P) + sub) * P
                        nc.sync.dma_start(out=hcat[n0:n0 + P, :], in_=ht2[:])
                nrows = NPAD // AG_CHUNKS
                nc.gpsimd.collective_compute(
                    "AllGather", mybir.AluOpType.bypass,
                    ins=[hcat[agc * nrows:(agc + 1) * nrows, :].opt()],
                    outs=[ag_bufs[agc][:].opt()],
                    replica_groups=[list(range(N_CORES))])

            # ---- metadata (resident) ----
            mds_t = mpool.tile([P, TB], f32)
            nc.sync.dma_start(out=mds_t[:], in_=mds_in[:])
            mval_t = mpool.tile([P, TB], f32)
            nc.sync.dma_start(out=mval_t[:], in_=mval_in[:])

            nreg_cache = {}

            def count_reg(v):
                if v not in nreg_cache:
                    nreg_cache[v] = nc.gpsimd.to_reg(v)
                return nreg_cache[v]

            # ---- phase 3: transposed aggregation ----
            # sweeps[i] = (q_lo, q_hi, mode): "full" closes the block in one
            # pass; "partial" parks raw agg^T in hgT; "final" adds the parked
            # partial back and applies PReLU.
            if AG_CHUNKS == 1:
                sweeps = [(0, NQ, "full")]
            else:
                sweeps = [(0, 2, "partial"), (2, 4, "final")]
            for q_lo, q_hi, mode in sweeps:
                for b in range(NB):
                    s0 = int(bstart[b])
                    koff0 = int(kb[b, :q_lo].sum())
                    nbatch = int(kb[b, q_lo:q_hi].sum())
                    if nbatch == 0:
                        continue
                    m0 = s0 + koff0           # first meta/batch column
                    it = ipool.tile([P, nbatch * 8], i16, tag="idx",
                                    name=f"idx{mode[0]}{b}")
                    nc.sync.dma_start(out=it[:],
                                      in_=idx_in[:, m0 * 8:(m0 + nbatch) * 8])
                    buckets = []          # (tile, koff, kbr)
                    koff = 0
                    for qq in range(q_lo, q_hi):
                        kbr = int(kb[b, qq])
                        if kbr == 0:
                            continue
                        if AG_CHUNKS == 1:
                            src = ag_bufs[0][qq * QROWS:(qq + 1) * QROWS, :]
                        else:
                            src = ag_bufs[qq >> 1][(qq & 1) * QROWS:
                                                   ((qq & 1) + 1) * QROWS, :]
                        gt = gpool.tile([P, kbr * H2], fp8, tag="g",
                                        name=f"g{mode[0]}{b}_{qq}")
                        nc.gpsimd.dma_gather(
                            out_ap=gt[:].rearrange("p (k h) -> p k h", k=kbr),
                            in_ap=src,
                            idxs_ap=it[:, koff * 8:(koff + kbr) * 8],
                            num_idxs=kbr * P,
                            num_idxs_reg=count_reg(kbr * P),
                            elem_size=H2,
                            single_packet=False)
                        if debug_outs and b == 1 and AG_CHUNKS == 1:
                            nc.sync.dma_start(
                                out=gt_out[:, qq * H2:(qq + 1) * H2],
                                in_=gt[:, :H2])
                        buckets.append((gt, koff, kbr))
                        koff += kbr
                    pT = [psp.tile([P, P], f32, space="PSUM", tag=f"b{cch}",
                                   name=f"pT{mode[0]}{b}_{cch}")
                          for cch in range(4)]
                    for gt, koff, kbr in buckets:
                        for jj in range(kbr):
                            j = koff + jj
                            s_t = spool.tile([P, P], bf16, tag="s")
                            nc.vector.tensor_scalar(
                                out=s_t[:], in0=iota_t[:],
                                scalar1=mds_t[:, m0 + j:m0 + j + 1],
                                scalar2=mval_t[:, m0 + j:m0 + j + 1],
                                op0=mybir.AluOpType.is_equal,
                                op1=mybir.AluOpType.mult)
                            for cch in range(4):
                                nc.tensor.matmul(
                                    pT[cch][:],
                                    lhsT=gt[:, jj * H2 + cch * P:
                                            jj * H2 + (cch + 1) * P],
                                    rhs=s_t[:],
                                    start=(j == 0), stop=(j == nbatch - 1))
                    for cch in range(4):
                        dst = hgT[:, b * H2 + cch * P:b * H2 + (cch + 1) * P]
                        if mode == "partial":
                            nc.scalar.activation(
                                out=dst, in_=pT[cch][:],
                                func=mybir.ActivationFunctionType.Copy)
                            continue
                        if mode == "final":
                            sum_t = hpool.tile([P, P], bf16, tag="sum")
                            nc.vector.tensor_tensor(
                                out=sum_t[:], in0=pT[cch][:], in1=dst,
                                op=mybir.AluOpType.add)
                            src_ap = sum_t[:]
                        else:
                            src_ap = pT[cch][:]
                        kwargs = {}
                        if cch < 2:
                            kwargs["accum_out"] = accA[:, cch * NB + b:
                                                       cch * NB + b + 1]
                        nc.scalar.activation(
                            out=dst, in_=src_ap,
                            func=mybir.ActivationFunctionType.Prelu,
                            alpha=alpha_t[:, :1], **kwargs)

            # ---- phase 3.5: s = sigmoid(mean(h1_gcn)); v = bilT @ s ----
            cs_t = hpool.tile([P, 2], f32, tag="cs")
            for cch in range(2):
                nc.vector.tensor_reduce(
                    out=cs_t[:, cch:cch + 1],
                    in_=accA[:, cch * NB:(cch + 1) * NB],
                    axis=mybir.AxisListType.X, op=mybir.AluOpType.add)
            nc.sync.dma_start(out=cs_in[:], in_=cs_t[:])
            nc.gpsimd.collective_compute(
                "AllReduce", mybir.AluOpType.add,
                ins=[cs_in[:].opt()], outs=[cs_out[:].opt()],
                replica_groups=[list(range(N_CORES))])
            cso_t = hpool.tile([P, 2], f32, tag="cso")
            nc.sync.dma_start(out=cso_t[:], in_=cs_out[:])
            sT_t = hpool.tile([P, 2], f32, tag="sT")
            nc.scalar.activation(out=sT_t[:], in_=cso_t[:],
                                 func=mybir.ActivationFunctionType.Sigmoid,
                                 scale=1.0 / N_NODES)
            vp = psp.tile([P, 2], f32, space="PSUM", tag="b3")
            for hc in range(2):
                for gc in range(2):
                    nc.tensor.matmul(
                        vp[:, hc:hc + 1],
                        lhsT=bilT_t[gc][:, hc * P:(hc + 1) * P],
                        rhs=sT_t[:, gc:gc + 1],
                        start=(gc == 0), stop=(gc == 1))
            vT_t = hpool.tile([P, 2], bf16, tag="vT")
            nc.vector.tensor_copy(out=vT_t[:], in_=vp[:])

            # ---- phase 4: scores via tiny matmuls into one PSUM ----
            psc = psp.tile([P, 2 * NB], f32, space="PSUM", tag="b2")
            for g2 in range(2):
                for b in range(NB):
                    for hc in range(2):
                        nc.tensor.matmul(
                            psc[:, g2 * NB + b:g2 * NB + b + 1],
                            lhsT=hgT[:, b * H2 + (2 * g2 + hc) * P:
                                     b * H2 + (2 * g2 + hc + 1) * P],
                            rhs=vT_t[:, hc:hc + 1],
                            start=(hc == 0), stop=(hc == 1))
            if debug_outs:
                nc.sync.dma_start(out=hcat_out[:], in_=hcat[:])
                if AG_CHUNKS == 1:
                    for qq in range(4):
                        nc.sync.dma_start(
                            out=hag_out[qq * P:(qq + 1) * P, :],
                            in_=ag_bufs[0][qq * QROWS + 512:
                                           qq * QROWS + 512 + P, :])
                nc.sync.dma_start(out=hgt_out[:], in_=hgT[:])
                nc.sync.dma_start(out=acc_out[:], in_=accA[:])

            sc_t = hpool.tile([P, 2 * NB], f32, tag="scb")
            nc.vector.tensor_scalar(
                out=sc_t[:], in0=psc[:], scalar1=bilb_t[:, :1],
                scalar2=None, op0=mybir.AluOpType.add)
            for g2 in range(2):
                nc.sync.dma_start(out=score_out[g2],
                                  in_=sc_t[:, g2 * NB:(g2 + 1) * NB])

    if lower:
        mybir.codegen_inst_isa_subclasses(nc)
        _split_multi_waits(nc)
    return nc


def kernel(x_1, x_2, edge_vals, fc_w, fc_b, prelu_a, bil_w, bil_b, edge_index):
    global LAST_EXEC_NS
    import hashlib
    ek = (hashlib.blake2b(np.ascontiguousarray(edge_index).tobytes(),
                          digest_size=16).digest(),
          hashlib.blake2b(np.ascontiguousarray(edge_vals).tobytes(),
                          digest_size=16).digest())
    if _CACHE.get("ekey") != ek:
        _CACHE.clear()
        _CACHE["ekey"] = ek
        _CACHE["pre"] = _preprocess_edges(edge_index, edge_vals)
    kb, nbb, TB, idx16, meta_ds, meta_val = _CACHE["pre"]

    with_bias = bool(np.any(np.asarray(fc_b)))
    pkey = ("prog", TB, with_bias, kb.tobytes())
    if pkey not in _CACHE:
        _CACHE[pkey] = _build_program(kb, nbb, TB, with_bias=with_bias)
    nc = _CACHE[pkey]
    global LAST_PROGRAM
    LAST_PROGRAM = nc

    bf = ml_dtypes.bfloat16
    x1 = np.asarray(x_1, np.float32).reshape(N_NODES, F)
    x2 = np.asarray(x_2, np.float32).reshape(N_NODES, F)
    wT = np.ascontiguousarray(np.asarray(fc_w, np.float32).T).astype(bf)
    bilT = np.ascontiguousarray(np.asarray(bil_w, np.float32)[0].T)
    fcb = np.asarray(fc_b, np.float32).reshape(1, H).astype(bf)
    iota = np.arange(P, dtype=np.float32).astype(bf)

    in_maps = []
    for c in range(N_CORES):
        xs = np.zeros((2, F, NPAD), bf)
        xs[0, :, :NPC] = x1[c * NPC:(c + 1) * NPC].T.astype(bf)
        xs[1, :, :NPC] = x2[c * NPC:(c + 1) * NPC].T.astype(bf)
        in_maps.append({
            "xT": xs,
            "wT": wT,
            "fcb": fcb,
            "alpha": np.asarray(prelu_a, np.float32).reshape(1),
            "bilT": bilT,
            "bilb": np.asarray(bil_b, np.float32).reshape(1),
            "iota": iota,
            "idx16": idx16[c],
            "mds": meta_ds[c],
            "mval": meta_val[c],
        })

    res = run_bass_kernel_spmd(nc, in_maps, list(range(N_CORES)))
    if res.exec_time_ns is not None:
        LAST_EXEC_NS = res.exec_time_ns

    out = np.empty((1, 2 * N_NODES), np.float32)
    for c in range(N_CORES):
        sc = res.results[c]["scores"]          # [2, 128, NB]
        out[0, c * NPC:(c + 1) * NPC] = sc[0].T.ravel()[:NPC]
        out[0, N_NODES + c * NPC:N_NODES + (c + 1) * NPC] = sc[1].T.ravel()[:NPC]
    return out
